# revision 31
# baseline (speedup 1.0000x reference)
"""ChebConv (K=3) forward as a distributed Bass/Tile kernel on 8 trn2 NeuronCores.

v5 structure (vertices V sharded across 8 cores, 98 blocks of 128 rows each):

  x0 = [x[0] | x[1]]                  # [V, 128], feature col = b*64 + fin
  x1 = L @ x0                         # phase 1: host-staged G1 stream
  z  = L @ x1                         # quarter-units: device gather from x1
  out = x0 (W0 - W2) + x1 W1 + z (2 W2) + bias     # folded Chebyshev mix

Phase 1 (SpMM1): host pre-gathers the source rows (G1 tape); the selector
tiles M1[e,l] = val*(l==lrow) are built on-chip by DVE tensor_scalar from a
compact (lrow,val) stream for most tiles, and streamed from DRAM for the
rest (keeps DVE off the critical path). One bf16 matmul per 128-edge tile
accumulates each row-block in a single-bank PSUM tile [P, 3P].

x1 row-blocks land in a persistent SBUF tile and per-quarter DRAM tensors;
four AllGathers (Shared outputs) fire as each vertex-quarter completes, with
an early small quarter so remote x1 becomes gatherable ~50us in.

Phase 2 is decomposed into independent (block-group, quarter) UNITS: the
unit's gather (SWDGE, 4 queues round-robin, indices resident in SBUF) feeds
matmuls accumulating z_q^T = (L_q x1)^T for 4 blocks in one PSUM bank; the
partial is written to a DRAM z-tape. Units are scheduled in quarter-major
order and INJECTED into the phase-1 instruction stream once their AllGather
and gather are predicted complete, so the descriptor-rate-limited SWDGE
stream runs continuously from ~50us instead of waiting for phase 1.

A final mix pass per 4-block group streams back the four z-tapes, transposes
x1 (PE), and evaluates bias + x0(W0-W2) + x1 W1 + sum_q z_q (2W2) in one
PSUM chain per block, storing [vpad, 128] f32 output (host de-interleaves).
"""

import sys

sys.path.insert(0, "/opt/trn_rl_repo")

import numpy as np
import ml_dtypes

import concourse.bass as bass
import concourse.bacc as bacc
import concourse.mybir as mybir
import concourse.tile as tile
from concourse import bass_utils

P = 128
F32 = mybir.dt.float32
BF16 = mybir.dt.bfloat16
I16 = mybir.dt.int16
NPBF16 = ml_dtypes.bfloat16
NQ = 4        # SWDGE queues (hardware max)
SB1 = 3       # phase-1 blocks per PSUM group
SB2 = 4       # phase-2 blocks per unit / mix group
GWCAP = 28    # max tiles per stream/gather run

# number of phase-1 M1 tiles built on DVE (rest streamed from DRAM);
# DVE builds ~930 tiles in ~320us which matches the phase-1 span
DVE_M1_TILES = 950
# units whose M2 is streamed (early, overlaps phase 1); later units use
# DVE builds once phase 1 stops using DVE
M2_STREAM_UNITS = 60

# injection schedule knobs (phase-1 groups ~10us each). Conservative: an
# injected unit whose gather is late stalls the in-order PE queue and
# feeds back into phase-1 throughput (observed in v5a).
INJ_START = 8          # first phase-1 group index that may host a unit
INJ_RATE = 1.0         # max units injected per phase-1 group beyond start
INJ_GATHER_RATE = 1.3  # estimated phase-1 groups per gather completion
INJ_AG_LAG = 5         # groups between AG emission and availability


def _cdiv(a, b):
    return -(-a // b)


# ---------------------------------------------------------------------------
# Host-side plan
# ---------------------------------------------------------------------------


class Plan:
    def __init__(self, V, ncores, rows, cols):
        assert V % ncores == 0
        self.V, self.ncores = V, ncores
        self.vsh = V // ncores
        self.nblk = _cdiv(self.vsh, P)               # 98
        self.vpad = self.nblk * P

        rows = np.asarray(rows, np.int64)
        cols = np.asarray(cols, np.int64)
        self.r_core = rows // self.vsh
        r_loc = rows - self.r_core * self.vsh
        self.blk = r_loc // P
        self.lrow = r_loc % P
        c_core = cols // self.vsh
        c_loc = cols - c_core * self.vsh
        self.srcblk = c_loc // P

        # ---- phase 1: per-block tiles -----------------------------------
        cnt1 = np.zeros((ncores, self.nblk), np.int64)
        np.add.at(cnt1, (self.r_core, self.blk), 1)
        T1 = _cdiv(np.max(cnt1, axis=0), P)
        T1 = np.maximum(T1, 1)
        self.T1 = T1
        self.base1 = np.concatenate(([0], np.cumsum(T1)))
        self.nt1 = int(self.base1[-1])
        self.tile1_block = np.repeat(np.arange(self.nblk), T1)
        self.tile1_start = np.zeros(self.nt1, bool)
        self.tile1_stop = np.zeros(self.nt1, bool)
        self.tile1_start[self.base1[:-1]] = True
        self.tile1_stop[self.base1[1:] - 1] = True

        # ---- quarter split search ---------------------------------------
        # first quarter small (early AG_0 -> early gather start), ramping up
        H = np.zeros((ncores, self.nblk, self.nblk), np.int32)
        np.add.at(H, (self.r_core, self.blk, self.srcblk), 1)
        C = np.zeros((ncores, self.nblk, self.nblk + 1), np.int64)
        C[:, :, 1:] = np.cumsum(H, axis=2)
        best = None
        for a in range(10, 19):
            for b in range(16, 29):
                for c in range(20, 33):
                    dd = self.nblk - a - b - c
                    if not (16 <= dd <= 32):
                        continue
                    bnd = [0, a, a + b, a + b + c, self.nblk]
                    D = C[:, :, bnd[1:]] - C[:, :, bnd[:-1]]
                    T = _cdiv(np.max(D, axis=0), P)
                    obj = (int(T.sum()), a)
                    if best is None or obj < best[0]:
                        best = (obj, bnd)
        self.qbnd = best[1]
        self.qblocks = [self.qbnd[i + 1] - self.qbnd[i] for i in range(4)]
        self.qrows = [qb * P for qb in self.qblocks]

        # ---- phase 2: (block, quarter) slots ----------------------------
        qidx = np.zeros(self.nblk, np.int64)
        for q in range(4):
            qidx[self.qbnd[q]:self.qbnd[q + 1]] = q
        self.q2 = qidx[self.srcblk]
        qlo_rows = np.array([self.qbnd[q] * P for q in range(4)])
        qrows_arr = np.array(self.qrows)
        self.fq = (c_core * qrows_arr[self.q2]
                   + (c_loc - qlo_rows[self.q2])).astype(np.int64)
        assert self.fq.max() < 32768

        # slot order: for group(SB2): for q: for block in group
        self.ng2 = _cdiv(self.nblk, SB2)
        slot_of = np.zeros((self.nblk, 4), np.int64)
        order = []
        for g in range(self.ng2):
            b0 = g * SB2
            bh = min(SB2, self.nblk - b0)
            for q in range(4):
                for bi in range(bh):
                    slot_of[b0 + bi, q] = len(order)
                    order.append((b0 + bi, q))
        self.nslots = len(order)
        self.slot_block = np.array([b for b, _ in order])
        self.slot_q = np.array([q for _, q in order])
        self.sid2 = slot_of[self.blk, self.q2]

        cnt2 = np.zeros((ncores, self.nslots), np.int64)
        np.add.at(cnt2, (self.r_core, self.sid2), 1)
        T2 = _cdiv(np.max(cnt2, axis=0), P)
        blk_tiles = np.zeros(self.nblk, np.int64)
        np.add.at(blk_tiles, self.slot_block, T2)
        for b in np.nonzero(blk_tiles == 0)[0]:
            T2[slot_of[b, 0]] = 1
        self.T2 = T2
        self.base2 = np.concatenate(([0], np.cumsum(T2)))
        self.nt2 = int(self.base2[-1])
        self.tile2_block = np.repeat(self.slot_block, T2)
        # start/stop per SLOT (independent (block, quarter) partials)
        self.tile2_start = np.zeros(self.nt2, bool)
        self.tile2_stop = np.zeros(self.nt2, bool)
        nz = np.nonzero(T2)[0]
        self.tile2_start[self.base2[nz]] = True
        self.tile2_stop[self.base2[nz + 1] - 1] = True
        # per (block, q): any tiles?
        self.has_zq = np.zeros((self.nblk, 4), bool)
        for s in nz:
            self.has_zq[self.slot_block[s], self.slot_q[s]] = True

        # ---- runs -------------------------------------------------------
        # phase 1: per group(SB1), tile ranges split <= GWCAP
        self.nsb1 = _cdiv(self.nblk, SB1)
        self.runs1 = []
        for s in range(self.nsb1):
            b0 = s * SB1
            bh = min(SB1, self.nblk - b0)
            t0, t1 = int(self.base1[b0]), int(self.base1[b0 + bh])
            rr = []
            t = t0
            while t < t1:
                n = min(GWCAP, t1 - t)
                rr.append((t, n))
                t += n
            self.runs1.append(rr)
        # phase 2 units: (g, q) -> list of (t0, ntr) pieces (<= GWCAP)
        self.units = []  # (g, q, [(t0, ntr), ...])
        s = 0
        for g in range(self.ng2):
            b0 = g * SB2
            bh = min(SB2, self.nblk - b0)
            for q in range(4):
                t0 = int(self.base2[s])
                ntr = int(np.sum(T2[s:s + bh]))
                if ntr > 0:
                    pieces = []
                    t = t0
                    while t < t0 + ntr:
                        n = min(GWCAP, t0 + ntr - t)
                        pieces.append((t, n))
                        t += n
                    self.units.append((g, q, pieces))
                s += bh
        # quarter-major unit order (gather/consumption order)
        self.unit_order = sorted(
            range(len(self.units)), key=lambda u: (self.units[u][1],
                                                   self.units[u][0])
        )
        self.GW = max(
            max(n for rr in self.runs1 for _, n in rr),
            max(n for _, _, pp in self.units for _, n in pp),
        )
        self.ntiles = self.nt2

    # ---- per-core content arrays ---------------------------------------
    def per_core_arrays(self, core, vals, x0bf):
        sel = np.nonzero(self.r_core == core)[0]

        # phase 1 (sorted by dest block)
        sid = self.blk[sel]
        o = np.argsort(sid, kind="stable")
        s1, sid1 = sel[o], sid[o]
        start = np.searchsorted(sid1, np.arange(self.nblk))
        rank = np.arange(len(sid1)) - start[sid1]
        pos = self.base1[sid1] * P + rank
        n1 = self.nt1 * P
        g1idx = np.zeros(n1, np.int64)
        g1idx[pos] = np.asarray(self.colsg[s1])
        g1 = np.ascontiguousarray(
            x0bf[g1idx].reshape(self.nt1, P, P).transpose(1, 0, 2)
            .reshape(P, self.nt1 * P)
        )
        lrow_col = np.zeros(n1, np.float32)
        val_col = np.zeros(n1, np.float32)
        lrow_col[pos] = self.lrow[s1]
        val_col[pos] = vals[s1]
        lv1 = np.zeros((P, self.nt1 * 2), np.float32)
        lv1[:, 0::2] = lrow_col.reshape(self.nt1, P).T
        lv1[:, 1::2] = val_col.reshape(self.nt1, P).T
        lv1 = np.ascontiguousarray(lv1)
        m1 = np.zeros((self.nt1, P, P), np.float32)
        m1[pos // P, pos % P, self.lrow[s1]] = vals[s1]
        m1 = np.ascontiguousarray(
            m1.astype(NPBF16).transpose(1, 0, 2).reshape(P, self.nt1 * P)
        )

        # phase 2 (sorted by slot)
        sid = self.sid2[sel]
        o = np.argsort(sid, kind="stable")
        s2, sid2 = sel[o], sid[o]
        start = np.searchsorted(sid2, np.arange(self.nslots))
        rank = np.arange(len(sid2)) - start[sid2]
        pos = self.base2[sid2] * P + rank
        n2 = self.nt2 * P
        idx = np.zeros(n2, np.int16)
        idx[pos] = self.fq[s2].astype(np.int16)
        idx_w = np.tile(np.ascontiguousarray(idx.reshape(-1, 16).T), (8, 1))
        m2 = np.zeros((self.nt2, P, P), np.float32)
        m2[pos // P, pos % P, self.lrow[s2]] = vals[s2]
        m2 = np.ascontiguousarray(
            m2.astype(NPBF16).transpose(1, 0, 2).reshape(P, self.nt2 * P)
        )
        lrow_col = np.zeros(n2, np.float32)
        val_col = np.zeros(n2, np.float32)
        lrow_col[pos] = self.lrow[s2]
        val_col[pos] = vals[s2]
        lv2 = np.zeros((P, self.nt2 * 2), np.float32)
        lv2[:, 0::2] = lrow_col.reshape(self.nt2, P).T
        lv2[:, 1::2] = val_col.reshape(self.nt2, P).T
        lv2 = np.ascontiguousarray(lv2)
        return g1, lv1, m1, idx_w, m2, lv2


# ---------------------------------------------------------------------------
# Bass program
# ---------------------------------------------------------------------------


def build_program(pl: Plan):
    nblk, ncores, GW = pl.nblk, pl.ncores, pl.GW

    nc = bacc.Bacc(
        "TRN2",
        target_bir_lowering=False,
        debug=False,
        num_devices=ncores,
        num_swdge_queues=NQ,
    )

    g1d = nc.dram_tensor("g1d", [P, pl.nt1 * P], BF16, kind="ExternalInput")
    lv1d = nc.dram_tensor("lv1d", [P, pl.nt1 * 2], F32, kind="ExternalInput")
    m1d = nc.dram_tensor("m1d", [P, pl.nt1 * P], BF16, kind="ExternalInput")
    m2d = nc.dram_tensor("m2d", [P, pl.nt2 * P], BF16, kind="ExternalInput")
    lv2d = nc.dram_tensor("lv2d", [P, pl.nt2 * 2], F32, kind="ExternalInput")
    eidx = nc.dram_tensor("eidx", [P, pl.nt2 * 8], I16, kind="ExternalInput")
    iota_d = nc.dram_tensor("iota", [P, P], BF16, kind="ExternalInput")
    x0td = nc.dram_tensor("x0t", [P, nblk * P], BF16, kind="ExternalInput")
    wbd = nc.dram_tensor("wbd", [P, 3 * P], BF16, kind="ExternalInput")
    biasbd = nc.dram_tensor("biasbd", [1, P], BF16, kind="ExternalInput")
    ident_d = nc.dram_tensor("ident", [P, P], BF16, kind="ExternalInput")
    ones_d = nc.dram_tensor("ones1", [1, P], BF16, kind="ExternalInput")
    outp = nc.dram_tensor("outp", [nblk, P, P], F32, kind="ExternalOutput")

    x1my = [
        nc.dram_tensor(f"x1my{q}", [pl.qblocks[q], P, P], BF16)
        for q in range(4)
    ]
    x1full = [
        nc.dram_tensor(
            f"x1full{q}", [ncores * pl.qrows[q], P], BF16, addr_space="Shared"
        )
        for q in range(4)
    ]
    # feature-major [f, b*P + l] so block-range DMA slices match the SBUF
    # staging tiles' flat iteration order
    ztape = [
        nc.dram_tensor(f"ztape{q}", [P, nblk * P], BF16) for q in range(4)
    ]

    uo = pl.unit_order
    nunits = len(uo)
    # per-unit M2 source: stream early units, DVE-build late ones
    m2_stream = [k < M2_STREAM_UNITS for k in range(nunits)]

    with tile.TileContext(nc) as tc:
        with (
            tc.tile_pool(name="const", bufs=1) as cpool,
            tc.tile_pool(name="x1res", bufs=1) as x1pool,
            tc.tile_pool(name="g1sl", bufs=3) as g1pool,
            tc.tile_pool(name="m1sl", bufs=5) as m1pool,
            tc.tile_pool(name="lvsl", bufs=5) as lvpool,
            tc.tile_pool(name="g2sl", bufs=7) as g2pool,
            tc.tile_pool(name="m2sl", bufs=3) as m2pool,
            tc.tile_pool(name="x0sl", bufs=2) as xpool,
            tc.tile_pool(name="z3st", bufs=2) as zspool,
            tc.tile_pool(name="zinl", bufs=4) as zipool,
            tc.tile_pool(name="x1tst", bufs=2) as x1tpool,
            tc.tile_pool(name="obst", bufs=2) as opool,
            tc.tile_pool(name="acc1", bufs=3, space="PSUM") as apool,
            tc.tile_pool(name="zacc", bufs=2, space="PSUM") as zpool,
            tc.tile_pool(name="ptr", bufs=1, space="PSUM") as ptpool,
            tc.tile_pool(name="pmix", bufs=2, space="PSUM") as pmpool,
        ):
            ident_s = cpool.tile([P, P], BF16, tag="ident")
            nc.sync.dma_start(out=ident_s[:], in_=ident_d[:, :])
            ones_s = cpool.tile([1, P], BF16, tag="ones")
            nc.sync.dma_start(out=ones_s[:], in_=ones_d[:, :])
            bias_s = cpool.tile([1, P], BF16, tag="bias")
            nc.sync.dma_start(out=bias_s[:], in_=biasbd[:, :])
            wbd_s = cpool.tile([P, 3 * P], BF16, tag="wbd")
            nc.sync.dma_start(out=wbd_s[:], in_=wbd[:, :])
            iota_s = cpool.tile([P, P], BF16, tag="iota")
            nc.sync.dma_start(out=iota_s[:], in_=iota_d[:, :])
            eidx_s = cpool.tile([P, pl.nt2 * 8], I16, tag="eidx")
            nc.sync.dma_start(out=eidx_s[:], in_=eidx[:, :])

            x1sb = x1pool.tile([P, nblk * P], BF16, tag="x1sb")

            # unit k -> list of g tiles (per piece). Gathers must FOLLOW
            # their quarter's AG in program order (RAW on x1full[q]), and
            # AG dispatches share the Pool queue with the gathers — so we
            # only keep a bounded number of unconsumed gathers ahead of
            # each AG dispatch to avoid buffer-stall head-of-line blocking.
            g2tiles = {}
            gather_emitted = [0]
            ag_group = [None] * 4

            def emit_gathers_until(limit):
                while gather_emitted[0] < min(limit, nunits):
                    k = gather_emitted[0]
                    _, q, pieces = pl.units[uo[k]]
                    if ag_group[q] is None:
                        return
                    tl = []
                    for (t0, ntr) in pieces:
                        gt = g2pool.tile([P, GW * P], BF16, tag="g2")
                        nidx = ntr * P
                        nc.gpsimd.dma_gather(
                            out_ap=gt[:, :nidx].rearrange(
                                "p (t e) -> p t e", e=P),
                            in_ap=x1full[q][:, :],
                            idxs_ap=eidx_s[:, t0 * 8:(t0 + ntr) * 8],
                            num_idxs=nidx,
                            num_idxs_reg=nidx,
                            elem_size=P,
                            single_packet=False,
                            queue_num=(k + len(tl)) % NQ,
                        )
                        tl.append(gt)
                    g2tiles[k] = tl
                    gather_emitted[0] += 1

            # ---- emission helpers ---------------------------------------
            def emit_phase1_group(s):
                b0 = s * SB1
                bh = min(SB1, nblk - b0)
                acc = apool.tile([P, SB1 * P], F32, tag="acc1",
                                 name=f"a1_{s}")
                for ri, (t0, ntr) in enumerate(pl.runs1[s]):
                    gt = g1pool.tile([P, GW * P], BF16, tag="g1")
                    eng = nc.sync if (ri % 2 == 0) else nc.scalar
                    eng.dma_start(
                        out=gt[:, :ntr * P],
                        in_=g1d[:, t0 * P:(t0 + ntr) * P],
                    )
                    if t0 + ntr <= DVE_M1_TILES:
                        lvt = lvpool.tile([P, GW * 2], F32, tag="lv")
                        nc.scalar.dma_start(
                            out=lvt[:, :ntr * 2],
                            in_=lv1d[:, t0 * 2:(t0 + ntr) * 2],
                        )
                        m = m1pool.tile([P, GW * P], BF16, tag="m1")
                        for tt in range(ntr):
                            nc.vector.tensor_scalar(
                                out=m[:, tt * P:(tt + 1) * P],
                                in0=iota_s[:],
                                scalar1=lvt[:, 2 * tt:2 * tt + 1],
                                scalar2=lvt[:, 2 * tt + 1:2 * tt + 2],
                                op0=mybir.AluOpType.is_equal,
                                op1=mybir.AluOpType.mult,
                            )
                    else:
                        m = m1pool.tile([P, GW * P], BF16, tag="m1")
                        nc.sync.dma_start(
                            out=m[:, :ntr * P],
                            in_=m1d[:, t0 * P:(t0 + ntr) * P],
                        )
                    for tt in range(ntr):
                        t = t0 + tt
                        bi = int(pl.tile1_block[t]) - b0
                        nc.tensor.matmul(
                            out=acc[:, bi * P:(bi + 1) * P],
                            lhsT=m[:, tt * P:(tt + 1) * P],
                            rhs=gt[:, tt * P:(tt + 1) * P],
                            start=bool(pl.tile1_start[t]),
                            stop=bool(pl.tile1_stop[t]),
                        )
                nc.scalar.copy(
                    out=x1sb[:, b0 * P:(b0 + bh) * P], in_=acc[:, :bh * P]
                )
                # stores split at quarter boundaries
                b = b0
                while b < b0 + bh:
                    q = 0
                    while pl.qbnd[q + 1] <= b:
                        q += 1
                    be = min(b0 + bh, pl.qbnd[q + 1])
                    for bb in range(b, be):
                        nc.sync.dma_start(
                            out=x1my[q][bb - pl.qbnd[q], :, :],
                            in_=x1sb[:, bb * P:(bb + 1) * P],
                        )
                    b = be

            def emit_ag(q):
                nc.gpsimd.collective_compute(
                    "AllGather",
                    mybir.AluOpType.bypass,
                    replica_groups=[list(range(ncores))],
                    ins=[x1my[q].ap().opt()],
                    outs=[x1full[q].ap().opt()],
                )

            def emit_unit(k):
                g, q, pieces = pl.units[uo[k]]
                b0 = g * SB2
                bh = min(SB2, nblk - b0)
                zacc = zpool.tile([P, SB2 * P], F32, tag="zacc",
                                  name=f"z_{k}")
                for pi, (t0, ntr) in enumerate(pieces):
                    gt = g2tiles[k][pi]
                    if m2_stream[k]:
                        m = m2pool.tile([P, GW * P], BF16, tag="m2")
                        nc.scalar.dma_start(
                            out=m[:, :ntr * P],
                            in_=m2d[:, t0 * P:(t0 + ntr) * P],
                        )
                    else:
                        lvt = lvpool.tile([P, GW * 2], F32, tag="lv")
                        nc.scalar.dma_start(
                            out=lvt[:, :ntr * 2],
                            in_=lv2d[:, t0 * 2:(t0 + ntr) * 2],
                        )
                        m = m2pool.tile([P, GW * P], BF16, tag="m2")
                        for tt in range(ntr):
                            nc.vector.tensor_scalar(
                                out=m[:, tt * P:(tt + 1) * P],
                                in0=iota_s[:],
                                scalar1=lvt[:, 2 * tt:2 * tt + 1],
                                scalar2=lvt[:, 2 * tt + 1:2 * tt + 2],
                                op0=mybir.AluOpType.is_equal,
                                op1=mybir.AluOpType.mult,
                            )
                    for tt in range(ntr):
                        t = t0 + tt
                        bi = int(pl.tile2_block[t]) - b0
                        nc.tensor.matmul(
                            out=zacc[:, bi * P:(bi + 1) * P],
                            lhsT=gt[:, tt * P:(tt + 1) * P],
                            rhs=m[:, tt * P:(tt + 1) * P],
                            start=bool(pl.tile2_start[t]),
                            stop=bool(pl.tile2_stop[t]),
                        )
                z3 = zspool.tile([P, SB2 * P], BF16, tag="z3")
                nc.scalar.copy(out=z3[:, :bh * P], in_=zacc[:, :bh * P])
                nc.sync.dma_start(
                    out=ztape[q][:, b0 * P:(b0 + bh) * P],
                    in_=z3[:, :bh * P],
                )

            def emit_mix(g):
                b0 = g * SB2
                bh = min(SB2, nblk - b0)
                x0sb = xpool.tile([P, SB2 * P], BF16, tag="x0sb")
                nc.sync.dma_start(
                    out=x0sb[:, :bh * P], in_=x0td[:, b0 * P:(b0 + bh) * P]
                )
                zins = []
                for q in range(4):
                    zi = zipool.tile([P, SB2 * P], BF16, tag="zin")
                    eng = nc.scalar if q % 2 else nc.sync
                    eng.dma_start(
                        out=zi[:, :bh * P],
                        in_=ztape[q][:, b0 * P:(b0 + bh) * P],
                    )
                    zins.append(zi)
                pt = ptpool.tile([P, SB2 * P], BF16, tag="ptr")
                for bi in range(bh):
                    nc.tensor.transpose(
                        out=pt[:, bi * P:(bi + 1) * P],
                        in_=x1sb[:, (b0 + bi) * P:(b0 + bi + 1) * P],
                        identity=ident_s[:],
                    )
                x1t = x1tpool.tile([P, SB2 * P], BF16, tag="x1t")
                nc.scalar.copy(out=x1t[:, :bh * P], in_=pt[:, :bh * P])
                pm = pmpool.tile([P, SB2 * P], F32, tag="pmix")
                for bi in range(bh):
                    b = b0 + bi
                    po = pm[:, bi * P:(bi + 1) * P]
                    nc.tensor.matmul(
                        out=po, lhsT=ones_s[:], rhs=bias_s[:],
                        start=True, stop=False,
                    )
                    nc.tensor.matmul(
                        out=po, lhsT=x0sb[:, bi * P:(bi + 1) * P],
                        rhs=wbd_s[:, 0:P], start=False, stop=False,
                    )
                    nc.tensor.matmul(
                        out=po, lhsT=x1t[:, bi * P:(bi + 1) * P],
                        rhs=wbd_s[:, P:2 * P], start=False, stop=False,
                    )
                    qs = [q for q in range(4) if pl.has_zq[b, q]]
                    for j, q in enumerate(qs):
                        nc.tensor.matmul(
                            out=po, lhsT=zins[q][:, bi * P:(bi + 1) * P],
                            rhs=wbd_s[:, 2 * P:3 * P],
                            start=False, stop=(j == len(qs) - 1),
                        )
                ob = opool.tile([P, SB2 * P], F32, tag="ob")
                nc.scalar.copy(out=ob[:, :bh * P], in_=pm[:, :bh * P])
                for bi in range(bh):
                    nc.sync.dma_start(
                        out=outp[b0 + bi, :, :],
                        in_=ob[:, bi * P:(bi + 1) * P],
                    )

            # ---- phase 1 with scheduled unit injection ------------------
            ag_next = 0
            injected = 0
            for s in range(pl.nsb1):
                emit_phase1_group(s)
                b_end = min((s + 1) * SB1, nblk)
                while ag_next < 4 and pl.qbnd[ag_next + 1] <= b_end:
                    emit_ag(ag_next)
                    ag_group[ag_next] = s
                    ag_next += 1
                    emit_gathers_until(injected + 4)
                # inject units whose AG + gather are predicted done
                while injected < nunits:
                    if injected >= INJ_RATE * max(0, s - INJ_START):
                        break
                    q = pl.units[uo[injected]][1]
                    if ag_group[q] is None:
                        break
                    g_ready = max(
                        ag_group[q] + INJ_AG_LAG,
                        INJ_START + int(INJ_GATHER_RATE * (injected + 1)),
                    )
                    if s < g_ready:
                        break
                    emit_gathers_until(injected + 4)
                    if gather_emitted[0] <= injected:
                        break
                    emit_unit(injected)
                    injected += 1
                emit_gathers_until(injected + 4)

            # all AGs dispatched; release the remaining gathers
            emit_gathers_until(nunits)

            # ---- remaining units + mixes --------------------------------
            mix_done = 0
            # mix group g ready when its q3 unit emitted; unit order is
            # quarter-major so q3 units come last, in group order
            for k in range(injected, nunits):
                emit_unit(k)
                while mix_done < pl.ng2:
                    # all units of group mix_done emitted?
                    pending = any(
                        kk >= k + 1
                        for kk in range(nunits)
                        if pl.units[uo[kk]][0] == mix_done
                    )
                    if pending:
                        break
                    emit_mix(mix_done)
                    mix_done += 1
            while mix_done < pl.ng2:
                emit_mix(mix_done)
                mix_done += 1

    nc.compile()
    return nc


# ---------------------------------------------------------------------------
# Host driver
# ---------------------------------------------------------------------------


def prepare(x, weight, bias, lap_vals, lap_rows, lap_cols, ncores=8):
    x = np.asarray(x, np.float32)
    weight = np.asarray(weight, np.float32)
    bias = np.asarray(bias, np.float32)
    lap_vals = np.asarray(lap_vals, np.float32)
    B, V, FIN = x.shape
    _, K, FOUT = weight.shape
    assert B == 2 and FIN == 64 and K == 3 and FOUT == 64

    pl = Plan(V, ncores, lap_rows, lap_cols)
    pl.colsg = np.asarray(lap_cols, np.int64)

    x0 = np.concatenate([x[0], x[1]], axis=1)
    x0bf = x0.astype(NPBF16)

    wk = [weight[:, k, :] for k in range(3)]
    wf = [wk[0] - wk[2], wk[1], 2.0 * wk[2]]
    wbd = np.zeros((P, 3 * P), np.float32)
    for k in range(3):
        wbd[:64, k * P:k * P + 64] = wf[k]
        wbd[64:, k * P + 64:k * P + 128] = wf[k]
    wbd = wbd.astype(NPBF16)
    biasbd = np.concatenate([bias, bias]).reshape(1, P).astype(NPBF16)
    ident = np.eye(P, dtype=np.float32).astype(NPBF16)
    ones1 = np.ones((1, P), NPBF16)
    iota_np = np.ascontiguousarray(
        np.tile(np.arange(P, dtype=np.float32), (P, 1)).astype(NPBF16)
    )

    in_maps = []
    for c in range(ncores):
        g1, lv1, m1, idx_w, m2, lv2 = pl.per_core_arrays(c, lap_vals, x0bf)
        sh = np.zeros((pl.vpad, P), NPBF16)
        sh[:pl.vsh] = x0bf[c * pl.vsh:(c + 1) * pl.vsh]
        x0t = np.ascontiguousarray(
            sh.reshape(pl.nblk, P, P).transpose(2, 0, 1).reshape(P, pl.nblk * P)
        )
        in_maps.append(
            {
                "g1d": g1,
                "lv1d": lv1,
                "m1d": m1,
                "m2d": m2,
                "lv2d": lv2,
                "eidx": idx_w,
                "iota": iota_np,
                "x0t": x0t,
                "wbd": wbd,
                "biasbd": biasbd,
                "ident": ident,
                "ones1": ones1,
            }
        )

    nc = build_program(pl)

    def assemble(results):
        out = np.empty((B, V, FOUT), np.float32)
        for c in range(ncores):
            o = np.asarray(results[c]["outp"]).reshape(pl.vpad, P)
            out[0, c * pl.vsh:(c + 1) * pl.vsh, :] = o[:pl.vsh, :64]
            out[1, c * pl.vsh:(c + 1) * pl.vsh, :] = o[:pl.vsh, 64:]
        return out

    return nc, in_maps, assemble, pl


def kernel(x, weight, bias, lap_vals, lap_rows, lap_cols):
    nc, in_maps, assemble, pl = prepare(
        x, weight, bias, lap_vals, lap_rows, lap_cols
    )
    res = bass_utils.run_bass_kernel_spmd(
        nc, in_maps, core_ids=list(range(pl.ncores))
    )
    return assemble(res.results)


# revision 33
# speedup vs baseline: 1.5671x; 1.5671x over previous
"""ChebConv (K=3) forward as a distributed Bass/Tile kernel on 8 trn2 NeuronCores.

v2 structure (vertices V sharded across 8 cores, 98 blocks of 128 rows each):

  x0 = [x[0] | x[1]]                  # [V, 128], feature col = b*64 + fin
  x1 = L @ x0                         # phase 1: fully HOST-STAGED streams
  z  = L @ x1                         # phase 2: device gather from AllGathered x1
  out = x0 (W0 - W2) + x1 W1 + z (2 W2) + bias     # folded Chebyshev mix

Phase 1 (SpMM1): the gather of x0 rows by edge source is precomputed on the
host (pure data movement), so the kernel streams two dense operand tapes
(G1 = gathered source rows, M1 = per-edge selector columns M[e, lrow]=val)
and runs one 128x128x128 bf16 matmul per 128-edge tile, accumulating each
row-block in PSUM. Tiles are packed per destination block (no chunking), so
padding is only the cross-core max of per-block degree.

x1 row-blocks are copied to a persistent SBUF tile (for phase-2 reuse) and
stored to per-quarter DRAM tensors. Four AllGathers (one per vertex-quarter,
Shared outputs) fire as soon as their quarter's blocks are done, so the
collective overlaps phase 1's tail and phase 2's head.

Phase 2 (SpMM2): per (sb, quarter) run, gpsimd.dma_gather fetches the 256B
x1 rows from the quarter's AllGathered table (int16 indices, 4 SWDGE queues
round-robin); matmul(lhsT=G, rhs=M) accumulates the TRANSPOSED block
z^T = (L x1)^T in PSUM. On block close the mix is fused: one PSUM chain of
bias (ones x bias outer product) + x0^T(W0-W2) + x1^T W1 + z^T(2W2) using
block-diagonal weights (both batches in one matmul), where x1^T comes from a
PE transpose of the SBUF-resident x1 block. Output [vpad, 128] f32 is
unsharded/de-interleaved on the host.
"""

import sys

sys.path.insert(0, "/opt/trn_rl_repo")

import numpy as np
import ml_dtypes

import concourse.bass as bass
import concourse.bacc as bacc
import concourse.mybir as mybir
import concourse.tile as tile
from concourse import bass_utils

P = 128
F32 = mybir.dt.float32
BF16 = mybir.dt.bfloat16
I16 = mybir.dt.int16
NPBF16 = ml_dtypes.bfloat16
NQ = 4        # SWDGE queues (hardware max)
SB = 3        # row-blocks per PSUM group
GWCAP = 26    # max tiles per stream/gather run


def _cdiv(a, b):
    return -(-a // b)


# ---------------------------------------------------------------------------
# Host-side plan: uniform (cross-core) tile structure from the edge data
# ---------------------------------------------------------------------------


class Plan:
    def __init__(self, V, ncores, rows, cols):
        assert V % ncores == 0
        self.V, self.ncores = V, ncores
        self.vsh = V // ncores                      # 12500
        self.nblk = _cdiv(self.vsh, P)              # 98
        self.vpad = self.nblk * P                   # 12544

        rows = np.asarray(rows, np.int64)
        cols = np.asarray(cols, np.int64)
        self.r_core = rows // self.vsh
        r_loc = rows - self.r_core * self.vsh
        self.blk = r_loc // P
        self.lrow = r_loc % P
        c_core = cols // self.vsh
        c_loc = cols - c_core * self.vsh
        self.srcblk = c_loc // P

        # ---- phase 1: per-block tiles -----------------------------------
        cnt1 = np.zeros((ncores, self.nblk), np.int64)
        np.add.at(cnt1, (self.r_core, self.blk), 1)
        T1 = _cdiv(np.max(cnt1, axis=0), P)
        T1 = np.maximum(T1, 1)
        self.T1 = T1
        self.base1 = np.concatenate(([0], np.cumsum(T1)))
        self.nt1 = int(self.base1[-1])
        tb = np.repeat(np.arange(self.nblk), T1)
        self.tile1_block = tb
        self.tile1_start = np.zeros(self.nt1, bool)
        self.tile1_stop = np.zeros(self.nt1, bool)
        self.tile1_start[self.base1[:-1]] = True
        self.tile1_stop[self.base1[1:] - 1] = True

        # ---- quarter split search (blocks per quarter, each <= 32) ------
        H = np.zeros((ncores, self.nblk, self.nblk), np.int32)
        np.add.at(H, (self.r_core, self.blk, self.srcblk), 1)
        C = np.zeros((ncores, self.nblk, self.nblk + 1), np.int64)
        C[:, :, 1:] = np.cumsum(H, axis=2)
        best = None
        for a in range(3, 33):
            for b in range(3, 33):
                for c in range(3, 33):
                    dd = self.nblk - a - b - c
                    if not (3 <= dd <= 20):  # small last quarter: early AG_3
                        continue
                    bnd = [0, a, a + b, a + b + c, self.nblk]
                    D = C[:, :, bnd[1:]] - C[:, :, bnd[:-1]]
                    T = _cdiv(np.max(D, axis=0), P)
                    obj = (int(T.sum()), dd)
                    if best is None or obj < best[0]:
                        best = (obj, bnd)
        self.qbnd = best[1]                          # block boundaries, len 5
        self.qblocks = [self.qbnd[i + 1] - self.qbnd[i] for i in range(4)]
        self.qrows = [qb * P for qb in self.qblocks]

        # ---- phase 2: (block, quarter) slots ----------------------------
        qidx = np.zeros(self.nblk, np.int64)
        for q in range(4):
            qidx[self.qbnd[q]:self.qbnd[q + 1]] = q
        self.q2 = qidx[self.srcblk]
        qlo_rows = np.array([self.qbnd[q] * P for q in range(4)])
        qrows_arr = np.array(self.qrows)
        self.fq = (c_core * qrows_arr[self.q2]
                   + (c_loc - qlo_rows[self.q2])).astype(np.int64)
        assert self.fq.max() < 32768

        # slot order: for sb: for q: for block in sb
        self.nsb = _cdiv(self.nblk, SB)
        slot_of = np.zeros((self.nblk, 4), np.int64)
        order = []
        for sb in range(self.nsb):
            b0 = sb * SB
            bh = min(SB, self.nblk - b0)
            for q in range(4):
                for bi in range(bh):
                    slot_of[b0 + bi, q] = len(order)
                    order.append((b0 + bi, q))
        self.nslots = len(order)
        self.slot_block = np.array([b for b, _ in order])
        self.slot_q = np.array([q for _, q in order])
        self.sid2 = slot_of[self.blk, self.q2]

        cnt2 = np.zeros((ncores, self.nslots), np.int64)
        np.add.at(cnt2, (self.r_core, self.sid2), 1)
        T2 = _cdiv(np.max(cnt2, axis=0), P)
        blk_tiles = np.zeros(self.nblk, np.int64)
        np.add.at(blk_tiles, self.slot_block, T2)
        for b in np.nonzero(blk_tiles == 0)[0]:
            T2[slot_of[b, 0]] = 1
        self.T2 = T2
        self.base2 = np.concatenate(([0], np.cumsum(T2)))
        self.nt2 = int(self.base2[-1])
        tile2_block = np.repeat(self.slot_block, T2)
        self.tile2_block = tile2_block
        self.tile2_start = np.zeros(self.nt2, bool)
        self.tile2_stop = np.zeros(self.nt2, bool)
        first, last = {}, {}
        for t in range(self.nt2):
            bb = int(tile2_block[t])
            if bb not in first:
                first[bb] = t
            last[bb] = t
        for t in first.values():
            self.tile2_start[t] = True
        for t in last.values():
            self.tile2_stop[t] = True

        # ---- runs -------------------------------------------------------
        # phase 1: per sb, tile range split into <= GWCAP pieces
        self.runs1 = []
        for sb in range(self.nsb):
            b0 = sb * SB
            bh = min(SB, self.nblk - b0)
            t0, t1 = int(self.base1[b0]), int(self.base1[b0 + bh])
            rr = []
            t = t0
            while t < t1:
                n = min(GWCAP, t1 - t)
                rr.append((t, n))
                t += n
            self.runs1.append(rr)
        # phase 2: per sb, one run per quarter (skip empty)
        self.runs2 = []
        s = 0
        for sb in range(self.nsb):
            b0 = sb * SB
            bh = min(SB, self.nblk - b0)
            rr = []
            for q in range(4):
                t0 = int(self.base2[s])
                ntr = int(np.sum(T2[s:s + bh]))
                if ntr > 0:
                    rr.append((t0, ntr, q))
                s += bh
            self.runs2.append(rr)
        self.GW = max(
            max(n for rr in self.runs1 for _, n in rr),
            max(n for rr in self.runs2 for _, n, _ in rr),
        )
        # compat with test harness prints
        self.ntiles = self.nt2

    # ---- per-core content arrays ---------------------------------------
    def per_core_arrays(self, core, vals, x0bf):
        sel = np.nonzero(self.r_core == core)[0]

        # phase 1 (sorted by dest block)
        sid = self.blk[sel]
        o = np.argsort(sid, kind="stable")
        s1, sid1 = sel[o], sid[o]
        start = np.searchsorted(sid1, np.arange(self.nblk))
        rank = np.arange(len(sid1)) - start[sid1]
        pos = self.base1[sid1] * P + rank
        n1 = self.nt1 * P
        g1idx = np.zeros(n1, np.int64)
        g1idx[pos] = np.asarray(self.colsg[s1])
        g1 = np.ascontiguousarray(
            x0bf[g1idx].reshape(self.nt1, P, P).transpose(1, 0, 2)
            .reshape(P, self.nt1 * P)
        )
        # compact (lrow, val) pairs for on-chip M1 build:
        # lv1[e, 2t] = lrow, lv1[e, 2t+1] = val (val=0 for padding)
        lrow_col = np.zeros(n1, np.float32)
        val_col = np.zeros(n1, np.float32)
        lrow_col[pos] = self.lrow[s1]
        val_col[pos] = vals[s1]
        lv1 = np.zeros((P, self.nt1 * 2), np.float32)
        lv1[:, 0::2] = lrow_col.reshape(self.nt1, P).T
        lv1[:, 1::2] = val_col.reshape(self.nt1, P).T
        lv1 = np.ascontiguousarray(lv1)  # f32: ALU scalars must be f32

        # phase 2 (sorted by slot)
        sid = self.sid2[sel]
        o = np.argsort(sid, kind="stable")
        s2, sid2 = sel[o], sid[o]
        start = np.searchsorted(sid2, np.arange(self.nslots))
        rank = np.arange(len(sid2)) - start[sid2]
        pos = self.base2[sid2] * P + rank
        n2 = self.nt2 * P
        idx = np.zeros(n2, np.int16)
        idx[pos] = self.fq[s2].astype(np.int16)
        idx_w = np.tile(np.ascontiguousarray(idx.reshape(-1, 16).T), (8, 1))
        m2 = np.zeros((self.nt2, P, P), np.float32)
        m2[pos // P, pos % P, self.lrow[s2]] = vals[s2]
        m2 = np.ascontiguousarray(
            m2.astype(NPBF16).transpose(1, 0, 2).reshape(P, self.nt2 * P)
        )
        return g1, lv1, idx_w, m2


# ---------------------------------------------------------------------------
# Bass program (SPMD: one program, per-core data via in_maps)
# ---------------------------------------------------------------------------


def build_program(pl: Plan):
    nblk, ncores, GW = pl.nblk, pl.ncores, pl.GW

    nc = bacc.Bacc(
        "TRN2",
        target_bir_lowering=False,
        debug=False,
        num_devices=ncores,
        num_swdge_queues=NQ,
    )

    g1d = nc.dram_tensor("g1d", [P, pl.nt1 * P], BF16, kind="ExternalInput")
    lv1d = nc.dram_tensor("lv1d", [P, pl.nt1 * 2], F32, kind="ExternalInput")
    iota_d = nc.dram_tensor("iota", [P, P], BF16, kind="ExternalInput")
    m2d = nc.dram_tensor("m2d", [P, pl.nt2 * P], BF16, kind="ExternalInput")
    eidx = nc.dram_tensor("eidx", [P, pl.nt2 * 8], I16, kind="ExternalInput")
    x0td = nc.dram_tensor("x0t", [P, nblk * P], BF16, kind="ExternalInput")
    wbd = nc.dram_tensor("wbd", [P, 3 * P], BF16, kind="ExternalInput")
    biasbd = nc.dram_tensor("biasbd", [1, P], BF16, kind="ExternalInput")
    ident_d = nc.dram_tensor("ident", [P, P], BF16, kind="ExternalInput")
    ones_d = nc.dram_tensor("ones1", [1, P], BF16, kind="ExternalInput")
    outp = nc.dram_tensor("outp", [nblk, P, P], F32, kind="ExternalOutput")

    x1my = [
        nc.dram_tensor(f"x1my{q}", [pl.qblocks[q], P, P], BF16)
        for q in range(4)
    ]
    x1full = [
        nc.dram_tensor(
            f"x1full{q}", [ncores * pl.qrows[q], P], BF16, addr_space="Shared"
        )
        for q in range(4)
    ]

    with tile.TileContext(nc) as tc:
        with (
            tc.tile_pool(name="const", bufs=1) as cpool,
            tc.tile_pool(name="x1res", bufs=1) as x1pool,
            tc.tile_pool(name="g1sl", bufs=4) as g1pool,
            tc.tile_pool(name="m1sl", bufs=5) as m1pool,
            tc.tile_pool(name="lv1sl", bufs=5) as lvpool,
            tc.tile_pool(name="g2sl", bufs=8) as g2pool,
            tc.tile_pool(name="m2sl", bufs=4) as m2pool,
            tc.tile_pool(name="x0sl", bufs=2) as xpool,
            tc.tile_pool(name="zst", bufs=4) as zpool,
            tc.tile_pool(name="x1tst", bufs=2) as x1tpool,
            tc.tile_pool(name="obst", bufs=4) as opool,
            tc.tile_pool(name="acc", bufs=5, space="PSUM") as apool,
            tc.tile_pool(name="ptr", bufs=1, space="PSUM") as ptpool,
            tc.tile_pool(name="pmix", bufs=2, space="PSUM") as pmpool,
        ):
            ident_s = cpool.tile([P, P], BF16, tag="ident")
            nc.sync.dma_start(out=ident_s[:], in_=ident_d[:, :])
            ones_s = cpool.tile([1, P], BF16, tag="ones")
            nc.sync.dma_start(out=ones_s[:], in_=ones_d[:, :])
            bias_s = cpool.tile([1, P], BF16, tag="bias")
            nc.sync.dma_start(out=bias_s[:], in_=biasbd[:, :])
            wbd_s = cpool.tile([P, 3 * P], BF16, tag="wbd")
            nc.sync.dma_start(out=wbd_s[:], in_=wbd[:, :])
            iota_s = cpool.tile([P, P], BF16, tag="iota")
            nc.sync.dma_start(out=iota_s[:], in_=iota_d[:, :])
            # resident gather-index table: frees the SP queue and removes
            # per-run idx loads from the gather dependency chain
            eidx_s = cpool.tile([P, pl.nt2 * 8], I16, tag="eidx")
            nc.sync.dma_start(out=eidx_s[:], in_=eidx[:, :])

            x1sb = x1pool.tile([P, nblk * P], BF16, tag="x1sb")

            # ---------------- phase 1: x1 = L @ x0 (streamed) ------------
            ag_next = 0
            for sb in range(pl.nsb):
                b0 = sb * SB
                bh = min(SB, nblk - b0)
                psums = {
                    b0 + bi: apool.tile([P, P], F32, tag="acc",
                                        name=f"a1_{b0 + bi}")
                    for bi in range(bh)
                }
                for ri1, (t0, ntr) in enumerate(pl.runs1[sb]):
                    g = g1pool.tile([P, GW * P], BF16, tag="g1")
                    # split the G1 stream across both HWDGE queues
                    eng = nc.sync if (ri1 % 2 == 0) else nc.scalar
                    eng.dma_start(
                        out=g[:, :ntr * P], in_=g1d[:, t0 * P:(t0 + ntr) * P]
                    )
                    lvt = lvpool.tile([P, GW * 2], F32, tag="lv1")
                    nc.scalar.dma_start(
                        out=lvt[:, :ntr * 2], in_=lv1d[:, t0 * 2:(t0 + ntr) * 2]
                    )
                    # build M1 tiles on DVE: M[e,l] = (iota[l]==lrow[e])*val[e]
                    m = m1pool.tile([P, GW * P], BF16, tag="m1")
                    for tt in range(ntr):
                        nc.vector.tensor_scalar(
                            out=m[:, tt * P:(tt + 1) * P],
                            in0=iota_s[:],
                            scalar1=lvt[:, 2 * tt:2 * tt + 1],
                            scalar2=lvt[:, 2 * tt + 1:2 * tt + 2],
                            op0=mybir.AluOpType.is_equal,
                            op1=mybir.AluOpType.mult,
                        )
                    for tt in range(ntr):
                        t = t0 + tt
                        b = int(pl.tile1_block[t])
                        nc.tensor.matmul(
                            out=psums[b][:],
                            lhsT=m[:, tt * P:(tt + 1) * P],
                            rhs=g[:, tt * P:(tt + 1) * P],
                            start=bool(pl.tile1_start[t]),
                            stop=bool(pl.tile1_stop[t]),
                        )
                for bi in range(bh):
                    b = b0 + bi
                    nc.scalar.copy(
                        out=x1sb[:, b * P:(b + 1) * P], in_=psums[b][:]
                    )
                    q = 0
                    while pl.qbnd[q + 1] <= b:
                        q += 1
                    nc.sync.dma_start(
                        out=x1my[q][b - pl.qbnd[q], :, :],
                        in_=x1sb[:, b * P:(b + 1) * P],
                    )
                # fire AllGathers for completed quarters
                while ag_next < 4 and pl.qbnd[ag_next + 1] <= b0 + bh:
                    q = ag_next
                    nc.gpsimd.collective_compute(
                        "AllGather",
                        mybir.AluOpType.bypass,
                        replica_groups=[list(range(ncores))],
                        ins=[x1my[q].ap().opt()],
                        outs=[x1full[q].ap().opt()],
                    )
                    ag_next += 1

            # ---------------- phase 2: z = L @ x1 + fused mix ------------
            # All gather calls are emitted FIRST so the Pool engine queue
            # holds nothing else: gathers self-pace on the 4 SWDGE queues,
            # prefetching into g2 buffers as soon as each AllGather lands.
            all_runs = [(sb, r) for sb in range(pl.nsb) for r in pl.runs2[sb]]
            g2tiles = []
            qload = [0] * NQ
            for k, (sb, (t0, ntr, q)) in enumerate(all_runs):
                g = g2pool.tile([P, GW * P], BF16, tag="g2")
                nidx = ntr * P
                qsel = min(range(NQ), key=lambda i: qload[i])
                qload[qsel] += ntr
                nc.gpsimd.dma_gather(
                    out_ap=g[:, :nidx].rearrange("p (t e) -> p t e", e=P),
                    in_ap=x1full[q][:, :],
                    idxs_ap=eidx_s[:, t0 * 8:(t0 + ntr) * 8],
                    num_idxs=nidx,
                    num_idxs_reg=nidx,
                    elem_size=P,
                    single_packet=False,
                    queue_num=qsel,
                )
                g2tiles.append(g)

            ri = 0
            for sb in range(pl.nsb):
                b0 = sb * SB
                bh = min(SB, nblk - b0)
                psums = {
                    b0 + bi: apool.tile([P, P], F32, tag="acc",
                                        name=f"a2_{b0 + bi}")
                    for bi in range(bh)
                }
                for (t0, ntr, q) in pl.runs2[sb]:
                    g = g2tiles[ri]
                    ri += 1
                    m = m2pool.tile([P, GW * P], BF16, tag="m2")
                    nc.scalar.dma_start(
                        out=m[:, :ntr * P], in_=m2d[:, t0 * P:(t0 + ntr) * P]
                    )
                    for tt in range(ntr):
                        t = t0 + tt
                        b = int(pl.tile2_block[t])
                        nc.tensor.matmul(
                            out=psums[b][:],
                            lhsT=g[:, tt * P:(tt + 1) * P],
                            rhs=m[:, tt * P:(tt + 1) * P],
                            start=bool(pl.tile2_start[t]),
                            stop=bool(pl.tile2_stop[t]),
                        )
                # block close: z^T in psum -> fused channel mix
                x0sb = xpool.tile([P, SB * P], BF16, tag="x0sb")
                nc.sync.dma_start(
                    out=x0sb[:, :bh * P],
                    in_=x0td[:, b0 * P:(b0 + bh) * P],
                )
                for bi in range(bh):
                    b = b0 + bi
                    z = zpool.tile([P, P], BF16, tag="z")
                    nc.scalar.copy(out=z[:], in_=psums[b][:])
                    pt = ptpool.tile([P, P], BF16, tag="ptr")
                    nc.tensor.transpose(
                        out=pt[:],
                        in_=x1sb[:, b * P:(b + 1) * P],
                        identity=ident_s[:],
                    )
                    x1t = x1tpool.tile([P, P], BF16, tag="x1t")
                    nc.scalar.copy(out=x1t[:], in_=pt[:])
                    pm = pmpool.tile([P, P], F32, tag="pmix", name=f"pm{b}")
                    nc.tensor.matmul(
                        out=pm[:], lhsT=ones_s[:], rhs=bias_s[:],
                        start=True, stop=False,
                    )
                    nc.tensor.matmul(
                        out=pm[:], lhsT=x0sb[:, bi * P:(bi + 1) * P],
                        rhs=wbd_s[:, 0:P], start=False, stop=False,
                    )
                    nc.tensor.matmul(
                        out=pm[:], lhsT=x1t[:],
                        rhs=wbd_s[:, P:2 * P], start=False, stop=False,
                    )
                    nc.tensor.matmul(
                        out=pm[:], lhsT=z[:],
                        rhs=wbd_s[:, 2 * P:3 * P], start=False, stop=True,
                    )
                    ob = opool.tile([P, P], F32, tag="ob")
                    nc.scalar.copy(out=ob[:], in_=pm[:])
                    nc.sync.dma_start(out=outp[b, :, :], in_=ob[:])

    nc.compile()
    return nc


# ---------------------------------------------------------------------------
# Host driver
# ---------------------------------------------------------------------------


def prepare(x, weight, bias, lap_vals, lap_rows, lap_cols, ncores=8):
    x = np.asarray(x, np.float32)
    weight = np.asarray(weight, np.float32)
    bias = np.asarray(bias, np.float32)
    lap_vals = np.asarray(lap_vals, np.float32)
    B, V, FIN = x.shape
    _, K, FOUT = weight.shape
    assert B == 2 and FIN == 64 and K == 3 and FOUT == 64

    pl = Plan(V, ncores, lap_rows, lap_cols)
    pl.colsg = np.asarray(lap_cols, np.int64)

    x0 = np.concatenate([x[0], x[1]], axis=1)          # [V, 128] f32
    x0bf = x0.astype(NPBF16)

    # folded block-diagonal weights: [W0-W2 | W1 | 2*W2]
    wk = [weight[:, k, :] for k in range(3)]
    wf = [wk[0] - wk[2], wk[1], 2.0 * wk[2]]
    wbd = np.zeros((P, 3 * P), np.float32)
    for k in range(3):
        wbd[:64, k * P:k * P + 64] = wf[k]
        wbd[64:, k * P + 64:k * P + 128] = wf[k]
    wbd = wbd.astype(NPBF16)
    biasbd = np.concatenate([bias, bias]).reshape(1, P).astype(NPBF16)
    ident = np.eye(P, dtype=np.float32).astype(NPBF16)
    ones1 = np.ones((1, P), NPBF16)
    iota_np = np.ascontiguousarray(
        np.tile(np.arange(P, dtype=np.float32), (P, 1)).astype(NPBF16)
    )

    in_maps = []
    for c in range(ncores):
        g1, lv1, idx_w, m2 = pl.per_core_arrays(c, lap_vals, x0bf)
        # x0^T blocks for the mix: x0t[f, b*128+l] = x0[core row b*128+l, f]
        sh = np.zeros((pl.vpad, P), NPBF16)
        sh[:pl.vsh] = x0bf[c * pl.vsh:(c + 1) * pl.vsh]
        x0t = np.ascontiguousarray(
            sh.reshape(pl.nblk, P, P).transpose(2, 0, 1).reshape(P, pl.nblk * P)
        )
        in_maps.append(
            {
                "g1d": g1,
                "lv1d": lv1,
                "iota": iota_np,
                "m2d": m2,
                "eidx": idx_w,
                "x0t": x0t,
                "wbd": wbd,
                "biasbd": biasbd,
                "ident": ident,
                "ones1": ones1,
            }
        )

    nc = build_program(pl)

    def assemble(results):
        out = np.empty((B, V, FOUT), np.float32)
        for c in range(ncores):
            o = np.asarray(results[c]["outp"]).reshape(pl.vpad, P)
            out[0, c * pl.vsh:(c + 1) * pl.vsh, :] = o[:pl.vsh, :64]
            out[1, c * pl.vsh:(c + 1) * pl.vsh, :] = o[:pl.vsh, 64:]
        return out

    return nc, in_maps, assemble, pl


def kernel(x, weight, bias, lap_vals, lap_rows, lap_cols):
    nc, in_maps, assemble, pl = prepare(
        x, weight, bias, lap_vals, lap_rows, lap_cols
    )
    res = bass_utils.run_bass_kernel_spmd(
        nc, in_maps, core_ids=list(range(pl.ncores))
    )
    return assemble(res.results)
